# revision 15
# baseline (speedup 1.0000x reference)
"""MinGRU Trainium2 kernel (bf16 data path, conv split PE/engines).

Reference computation (per batch b):
    c = depthwise_conv1d(x, conv_w, taps=5, pad=2)        # [D, L]
    h = h_w @ c                                           # [O, L]
    g = concat([-1000, +1000], g_w @ c)                   # [O, L]
    a = sigmoid(-g); out[l] = a[l]*out[l-1] + (1-a[l])*h[l]

Strategy: pure data-parallel over B (8 batches -> 8 NeuronCores), streamed
in 8 l-chunks of 512.  Everything below f32-PSUM precision runs in bf16
(x, weights, c, a, v-hat, out); the DVE scan keeps fp32 internal state so
bf16 operands only cost I/O rounding (~0.6% rel err total, gate is 2e-2).

Per chunk:
  - conv: d-tiles 0-2 as 5 diagonal bf16 matmuls on PE (PSUM f32), cast to
    bf16 SBUF on DVE/ACT; d-tile 3 on engines: 5 ACT scaled copies
    (per-partition conv weight as activation scale) + 4 GpSimd adds.
  - h/g 1x1 convs: bf16 matmuls on PE.
  - a = sigmoid(-(g+bias)) on ACT (bias rows 0/1 carry the -/+1000
    polarization; gw rows 0/1 are zero-padded).
  - vhat = (a-1)*h in ONE DVE scalar_tensor_tensor op (reads h from PSUM).
  - scan: out = a*prev - vhat on DVE (== a*prev + (1-a)*h).
    Channel 0 (a==1, vhat==0) stays exactly 0; channel 1 (a==0) becomes h;
    no special end-pass needed at the 2e-2 gate.
Output is stored bf16 and upcast to f32 on the host.
"""

import numpy as np
import ml_dtypes

import concourse.bass as bass
import concourse.mybir as mybir
from concourse import bacc
from concourse.tile import TileContext
from concourse.bass_utils import run_bass_kernel_spmd

BF16 = mybir.dt.bfloat16
F32 = mybir.dt.float32
AF = mybir.ActivationFunctionType
OP = mybir.AluOpType

B, D, O, L = 8, 512, 512, 4096
P = 128
CH = 512                 # l-chunk width (one PSUM bank)
NCH = L // CH            # 8
NDT = D // P             # 4 d-tiles
NOT = O // P             # 4 o-tiles
NTAPS = 5
N_CORES = 8
NPE = 4                  # conv d-tiles computed on PE; d-tile 3 on engines


def build_program():
    nc = bacc.Bacc()

    # x is host-padded with 2 zero columns on each side -> no halo cases
    x = nc.declare_dram_parameter("x", [D, L + 4], BF16, isOutput=False)
    hwT = nc.declare_dram_parameter("hwT", [D, O], BF16, isOutput=False)
    gwT = nc.declare_dram_parameter("gwT", [D, O], BF16, isOutput=False)
    cwdiag = nc.declare_dram_parameter("cwdiag", [NPE * P, NTAPS * P], BF16,
                                       isOutput=False)
    wtap = nc.declare_dram_parameter("wtap", [P, 2 * NTAPS], F32,
                                     isOutput=False)
    gbn = nc.declare_dram_parameter("gbn", [O, 1], F32, isOutput=False)
    zpad = nc.declare_dram_parameter("zpad", [P, 2], BF16, isOutput=False)
    out = nc.declare_dram_parameter("out", [O, L], BF16, isOutput=True)

    with TileContext(nc) as tc:
        with (
            tc.tile_pool(name="weights", bufs=1) as wpool,
            tc.tile_pool(name="xin", bufs=8) as xpool,
            tc.tile_pool(name="csb", bufs=8) as cpool,
            tc.tile_pool(name="mtap", bufs=10) as mpool,
            tc.tile_pool(name="actout", bufs=4) as apool,
            tc.tile_pool(name="vhat", bufs=4) as vpool,
            tc.tile_pool(name="outt", bufs=8) as opool,
            tc.tile_pool(name="cps", bufs=4, space="PSUM") as cps_pool,
            tc.tile_pool(name="hps", bufs=2, space="PSUM") as hps_pool,
            tc.tile_pool(name="gps", bufs=2, space="PSUM") as gps_pool,
        ):
            # tiny zero tile leads the scalar queue for PE warm-up matmuls
            warm_sb = wpool.tile([P, 2], BF16, tag="warm")
            nc.scalar.dma_start(out=warm_sb, in_=zpad[:, :])

            cw_sb = []
            for dt in range(NPE):
                t = wpool.tile([P, NTAPS * P], BF16, tag=f"cw{dt}")
                nc.scalar.dma_start(out=t, in_=cwdiag[dt * P:(dt + 1) * P, :])
                cw_sb.append(t)
            wtap_sb = wpool.tile([P, 2 * NTAPS], F32, tag="wtap")
            nc.scalar.dma_start(out=wtap_sb, in_=wtap[:, :])
            gbn_sb = []
            for ot in range(NOT):
                t = wpool.tile([P, 1], F32, tag=f"gbn{ot}")
                nc.scalar.dma_start(out=t, in_=gbn[ot * P:(ot + 1) * P, :])
                gbn_sb.append(t)

            # h/g weights: one coarse DMA per d-tile.  g weights go through
            # the GpSimd SWDGE queue (idle at preload time, few triggers so
            # chunk-0 conv adds aren't delayed); h weights on scalar HWDGE.
            hwT_sb, gwT_sb = [], []
            for dt in range(NDT):
                gt = wpool.tile([P, O], BF16, tag=f"gwT{dt}", name=f"gwT{dt}")
                nc.gpsimd.dma_start(out=gt, in_=gwT[dt * P:(dt + 1) * P, :])
                gwT_sb.append(gt)
            for dt in range(NDT):
                ht = wpool.tile([P, O], BF16, tag=f"hwT{dt}", name=f"hwT{dt}")
                nc.scalar.dma_start(out=ht, in_=hwT[dt * P:(dt + 1) * P, :])
                hwT_sb.append(ht)

            c_sb = [None] * NCH          # [chunk] -> list of 4 SBUF c tiles
            prev_out = [None] * NOT      # previous chunk's out tiles per ot

            def emit_conv(i):
                lo = i * CH
                tiles = []
                for dt in range(NDT):
                    xt = xpool.tile([P, CH + 4], BF16, tag="xt")
                    xq = nc.sync if dt % 2 == 0 else nc.scalar
                    xq.dma_start(out=xt[:, :],
                                 in_=x[dt * P:(dt + 1) * P, lo:lo + CH + 4])
                    if dt < NPE:
                        # PE path: 5 diagonal matmuls, center tap first
                        cp = cps_pool.tile([P, CH], F32, tag="cps")
                        tap_order = (2, 0, 1, 3, 4)
                        for j, k in enumerate(tap_order):
                            nc.tensor.matmul(
                                cp,
                                lhsT=cw_sb[dt][:, k * P:(k + 1) * P],
                                rhs=xt[:, k:k + CH],
                                start=(j == 0), stop=(j == NTAPS - 1),
                            )
                        ct = cpool.tile([P, CH], BF16, tag="ct")
                        if dt < 2:
                            nc.vector.tensor_copy(ct, cp)
                        else:
                            nc.scalar.copy(ct, cp)
                        tiles.append(ct)
                    else:
                        # engine path: ACT per-partition scaled copies + adds
                        base = 0 if dt == 3 else NTAPS
                        m = []
                        for k in range(NTAPS):
                            mt = mpool.tile([P, CH], BF16, tag="mt")
                            nc.scalar.activation(mt, xt[:, k:k + CH], AF.Copy,
                                                 bias=0.0,
                                                 scale=wtap_sb[:, base + k:
                                                               base + k + 1])
                            m.append(mt)
                        s01 = mpool.tile([P, CH], BF16, tag="s01")
                        nc.gpsimd.tensor_tensor(s01, m[0], m[1], OP.add)
                        s23 = mpool.tile([P, CH], BF16, tag="s23")
                        nc.gpsimd.tensor_tensor(s23, m[2], m[3], OP.add)
                        s014 = mpool.tile([P, CH], BF16, tag="s014")
                        nc.gpsimd.tensor_tensor(s014, s01, m[4], OP.add)
                        ct = cpool.tile([P, CH], BF16, tag="ct")
                        nc.gpsimd.tensor_tensor(ct, s014, s23, OP.add)
                        tiles.append(ct)
                c_sb[i] = tiles

            def emit_rest(i, last=False):
                lo = i * CH
                for ot in range(NOT):
                    # g before h: the sigmoid only needs g, so it starts
                    # while the h matmuls are still streaming
                    gp = gps_pool.tile([P, CH], F32, tag="gps")
                    for dt in range(NDT):
                        nc.tensor.matmul(
                            gp,
                            lhsT=gwT_sb[dt][:, ot * P:(ot + 1) * P],
                            rhs=c_sb[i][dt],
                            start=(dt == 0), stop=(dt == NDT - 1),
                        )
                    hp = hps_pool.tile([P, CH], F32, tag="hps")
                    for dt in range(NDT):
                        nc.tensor.matmul(
                            hp,
                            lhsT=hwT_sb[dt][:, ot * P:(ot + 1) * P],
                            rhs=c_sb[i][dt],
                            start=(dt == 0), stop=(dt == NDT - 1),
                        )
                    # a = sigmoid(-(g + bias))
                    at = apool.tile([P, CH], BF16, tag="at")
                    nc.scalar.activation(at, gp, AF.Sigmoid,
                                         bias=gbn_sb[ot], scale=-1.0)
                    # vhat = (a - 1) * h  == -(1-a)*h, h read from PSUM
                    nv = vpool.tile([P, CH], BF16, tag="nv")
                    nc.vector.scalar_tensor_tensor(nv, at, 1.0, hp,
                                                   OP.subtract, OP.mult)
                    # out = a*prev - vhat
                    ott = opool.tile([P, CH], BF16, tag=f"out{ot}")
                    init = 0.0 if i == 0 else prev_out[ot][:, CH - 1:CH]
                    nc.vector.tensor_tensor_scan(ott, at, nv, init,
                                                 OP.mult, OP.subtract)
                    if last:
                        # short transfers drain the tail faster
                        nc.sync.dma_start(
                            out=out[ot * P:(ot + 1) * P, lo:lo + CH // 2],
                            in_=ott[:, :CH // 2])
                        nc.sync.dma_start(
                            out=out[ot * P:(ot + 1) * P,
                                    lo + CH // 2:lo + CH],
                            in_=ott[:, CH // 2:])
                    else:
                        nc.sync.dma_start(
                            out=out[ot * P:(ot + 1) * P, lo:lo + CH],
                            in_=ott)
                    prev_out[ot] = ott

            # PE warm-up: dummy matmuls during the initial DMA wait trip the
            # HAM clock gate to 2.4 GHz before real work arrives; the result
            # (zeros) lands in an out region the chunk-0 store overwrites.
            wps = cps_pool.tile([P, CH], F32, tag="cps", name="warmps")
            for _ in range(28):
                nc.tensor.matmul(wps[0:2, 0:2], lhsT=warm_sb, rhs=warm_sb,
                                 start=True, stop=True)
            wout = wpool.tile([2, 2], BF16, tag="warmout")
            nc.vector.tensor_copy(wout, wps[0:2, 0:2])
            nc.gpsimd.dma_start(out=out[2:4, 0:2], in_=wout)

            # chunks, software-pipelined one chunk ahead
            emit_conv(0)
            emit_conv(1)
            for i in range(NCH - 2):
                emit_rest(i)
                emit_conv(i + 2)
            emit_rest(NCH - 2)
            emit_rest(NCH - 1, last=True)

    nc.finalize()
    return nc


_PROGRAM = None


def _get_program():
    global _PROGRAM
    if _PROGRAM is None:
        _PROGRAM = build_program()
    return _PROGRAM


def prepare_in_maps(x, conv_w, h_w, g_w):
    bf = ml_dtypes.bfloat16
    x = np.asarray(x, dtype=np.float32)
    conv_w = np.asarray(conv_w, dtype=np.float32)
    h_w = np.asarray(h_w, dtype=np.float32)
    g_w = np.asarray(g_w, dtype=np.float32)

    xpad = np.zeros((B, D, L + 4), np.float32)
    xpad[:, :, 2:L + 2] = x
    xpad = np.ascontiguousarray(xpad.astype(bf))

    hwT = np.ascontiguousarray(h_w[:, :, 0].T.astype(bf))         # [D, O]
    gw_pad = np.zeros((O, D), np.float32)
    gw_pad[2:, :] = g_w[:, :, 0]
    gwT = np.ascontiguousarray(gw_pad.T.astype(bf))               # [D, O]

    # diagonal conv-weight matrices for the PE d-tiles: [NPE*128, 5*128]
    cwdiag = np.zeros((NPE * P, NTAPS * P), np.float32)
    for dt in range(NPE):
        for k in range(NTAPS):
            blk = cwdiag[dt * P:(dt + 1) * P, k * P:(k + 1) * P]
            np.fill_diagonal(blk, conv_w[dt * P:(dt + 1) * P, 0, k])
    cwdiag = np.ascontiguousarray(cwdiag.astype(bf))

    # per-partition tap weights for the engine d-tiles (ACT scale input):
    # cols 0-4 for d-tile 3, cols 5-9 for d-tile 2 (odd chunks)
    wtap = np.ascontiguousarray(np.concatenate(
        [conv_w[3 * P:4 * P, 0, :], conv_w[2 * P:3 * P, 0, :]], axis=1))

    gbp = np.zeros((O, 1), np.float32)
    gbp[0, 0], gbp[1, 0] = -1000.0, 1000.0
    gbn = np.ascontiguousarray(-gbp)

    zpad = np.zeros((P, 2), bf)
    return [
        {"x": xpad[b], "hwT": hwT, "gwT": gwT, "cwdiag": cwdiag,
         "wtap": wtap, "gbn": gbn, "zpad": zpad}
        for b in range(B)
    ]


def kernel(x, conv_w, h_w, g_w):
    in_maps = prepare_in_maps(x, conv_w, h_w, g_w)
    nc = _get_program()
    res = run_bass_kernel_spmd(nc, in_maps, list(range(N_CORES))).results
    return np.stack([res[b]["out"].astype(np.float32) for b in range(B)],
                    axis=0)


# revision 16
# speedup vs baseline: 1.0642x; 1.0642x over previous
"""MinGRU Trainium2 kernel (bf16 data path, conv split PE/engines).

Reference computation (per batch b):
    c = depthwise_conv1d(x, conv_w, taps=5, pad=2)        # [D, L]
    h = h_w @ c                                           # [O, L]
    g = concat([-1000, +1000], g_w @ c)                   # [O, L]
    a = sigmoid(-g); out[l] = a[l]*out[l-1] + (1-a[l])*h[l]

Strategy: pure data-parallel over B (8 batches -> 8 NeuronCores), streamed
in 8 l-chunks of 512.  Everything below f32-PSUM precision runs in bf16
(x, weights, c, a, v-hat, out); the DVE scan keeps fp32 internal state so
bf16 operands only cost I/O rounding (~0.6% rel err total, gate is 2e-2).

Per chunk:
  - conv: d-tiles 0-2 as 5 diagonal bf16 matmuls on PE (PSUM f32), cast to
    bf16 SBUF on DVE/ACT; d-tile 3 on engines: 5 ACT scaled copies
    (per-partition conv weight as activation scale) + 4 GpSimd adds.
  - h/g 1x1 convs: bf16 matmuls on PE.
  - a = sigmoid(-(g+bias)) on ACT (bias rows 0/1 carry the -/+1000
    polarization; gw rows 0/1 are zero-padded).
  - vhat = (a-1)*h in ONE DVE scalar_tensor_tensor op (reads h from PSUM).
  - scan: out = a*prev - vhat on DVE (== a*prev + (1-a)*h).
    Channel 0 (a==1, vhat==0) stays exactly 0; channel 1 (a==0) becomes h;
    no special end-pass needed at the 2e-2 gate.
Output is stored bf16 and upcast to f32 on the host.
"""

import numpy as np
import ml_dtypes

import concourse.bass as bass
import concourse.mybir as mybir
from concourse import bacc
from concourse.tile import TileContext
from concourse.bass_utils import run_bass_kernel_spmd

BF16 = mybir.dt.bfloat16
F32 = mybir.dt.float32
AF = mybir.ActivationFunctionType
OP = mybir.AluOpType

B, D, O, L = 8, 512, 512, 4096
P = 128
CH = 512                 # l-chunk width (one PSUM bank)
NCH = L // CH            # 8
NDT = D // P             # 4 d-tiles
NOT = O // P             # 4 o-tiles
NTAPS = 5
N_CORES = 8
NPE = 3                  # conv d-tiles computed on PE; d-tile 3 on engines


def build_program():
    nc = bacc.Bacc()

    # x is host-padded with 2 zero columns on each side -> no halo cases
    x = nc.declare_dram_parameter("x", [D, L + 4], BF16, isOutput=False)
    hwT = nc.declare_dram_parameter("hwT", [D, O], BF16, isOutput=False)
    gwT = nc.declare_dram_parameter("gwT", [D, O], BF16, isOutput=False)
    cwdiag = nc.declare_dram_parameter("cwdiag", [NPE * P, NTAPS * P], BF16,
                                       isOutput=False)
    wtap = nc.declare_dram_parameter("wtap", [P, 2 * NTAPS], F32,
                                     isOutput=False)
    gbn = nc.declare_dram_parameter("gbn", [O, 1], F32, isOutput=False)
    zpad = nc.declare_dram_parameter("zpad", [P, 2], BF16, isOutput=False)
    out = nc.declare_dram_parameter("out", [O, L], BF16, isOutput=True)

    with TileContext(nc) as tc:
        with (
            tc.tile_pool(name="weights", bufs=1) as wpool,
            tc.tile_pool(name="xin", bufs=8) as xpool,
            tc.tile_pool(name="csb", bufs=8) as cpool,
            tc.tile_pool(name="mtap", bufs=10) as mpool,
            tc.tile_pool(name="actout", bufs=4) as apool,
            tc.tile_pool(name="vhat", bufs=4) as vpool,
            tc.tile_pool(name="outt", bufs=8) as opool,
            tc.tile_pool(name="cps", bufs=4, space="PSUM") as cps_pool,
            tc.tile_pool(name="hps", bufs=2, space="PSUM") as hps_pool,
            tc.tile_pool(name="gps", bufs=2, space="PSUM") as gps_pool,
        ):
            # tiny zero tile leads the scalar queue for PE warm-up matmuls
            warm_sb = wpool.tile([P, 2], BF16, tag="warm")
            nc.scalar.dma_start(out=warm_sb, in_=zpad[:, :])

            cw_sb = []
            for dt in range(NPE):
                t = wpool.tile([P, NTAPS * P], BF16, tag=f"cw{dt}")
                nc.scalar.dma_start(out=t, in_=cwdiag[dt * P:(dt + 1) * P, :])
                cw_sb.append(t)
            wtap_sb = wpool.tile([P, 2 * NTAPS], F32, tag="wtap")
            nc.scalar.dma_start(out=wtap_sb, in_=wtap[:, :])
            gbn_sb = []
            for ot in range(NOT):
                t = wpool.tile([P, 1], F32, tag=f"gbn{ot}")
                nc.scalar.dma_start(out=t, in_=gbn[ot * P:(ot + 1) * P, :])
                gbn_sb.append(t)

            # h/g weights: one coarse DMA per d-tile.  g weights go through
            # the GpSimd SWDGE queue (idle at preload time, few triggers so
            # chunk-0 conv adds aren't delayed); h weights on scalar HWDGE.
            hwT_sb, gwT_sb = [], []
            for dt in range(NDT):
                gt = wpool.tile([P, O], BF16, tag=f"gwT{dt}", name=f"gwT{dt}")
                nc.gpsimd.dma_start(out=gt, in_=gwT[dt * P:(dt + 1) * P, :])
                gwT_sb.append(gt)
            for dt in range(NDT):
                ht = wpool.tile([P, O], BF16, tag=f"hwT{dt}", name=f"hwT{dt}")
                nc.scalar.dma_start(out=ht, in_=hwT[dt * P:(dt + 1) * P, :])
                hwT_sb.append(ht)

            c_sb = [None] * NCH          # [chunk] -> list of 4 SBUF c tiles
            prev_out = [None] * NOT      # previous chunk's out tiles per ot

            def emit_conv(i):
                lo = i * CH
                tiles = []
                for dt in range(NDT):
                    xt = xpool.tile([P, CH + 4], BF16, tag="xt")
                    xq = nc.sync if dt % 2 == 0 else nc.scalar
                    xq.dma_start(out=xt[:, :],
                                 in_=x[dt * P:(dt + 1) * P, lo:lo + CH + 4])
                    if dt < NPE:
                        # PE path: 5 diagonal matmuls, center tap first
                        cp = cps_pool.tile([P, CH], F32, tag="cps")
                        tap_order = (2, 0, 1, 3, 4)
                        for j, k in enumerate(tap_order):
                            nc.tensor.matmul(
                                cp,
                                lhsT=cw_sb[dt][:, k * P:(k + 1) * P],
                                rhs=xt[:, k:k + CH],
                                start=(j == 0), stop=(j == NTAPS - 1),
                            )
                        ct = cpool.tile([P, CH], BF16, tag="ct")
                        if dt < 2:
                            nc.vector.tensor_copy(ct, cp)
                        else:
                            nc.scalar.copy(ct, cp)
                        tiles.append(ct)
                    else:
                        # engine path: ACT per-partition scaled copies + adds
                        base = 0 if dt == 3 else NTAPS
                        m = []
                        for k in range(NTAPS):
                            mt = mpool.tile([P, CH], BF16, tag="mt")
                            nc.scalar.activation(mt, xt[:, k:k + CH], AF.Copy,
                                                 bias=0.0,
                                                 scale=wtap_sb[:, base + k:
                                                               base + k + 1])
                            m.append(mt)
                        s01 = mpool.tile([P, CH], BF16, tag="s01")
                        nc.gpsimd.tensor_tensor(s01, m[0], m[1], OP.add)
                        s23 = mpool.tile([P, CH], BF16, tag="s23")
                        nc.gpsimd.tensor_tensor(s23, m[2], m[3], OP.add)
                        s014 = mpool.tile([P, CH], BF16, tag="s014")
                        nc.gpsimd.tensor_tensor(s014, s01, m[4], OP.add)
                        ct = cpool.tile([P, CH], BF16, tag="ct")
                        nc.gpsimd.tensor_tensor(ct, s014, s23, OP.add)
                        tiles.append(ct)
                c_sb[i] = tiles

            def emit_rest(i, last=False):
                lo = i * CH
                for ot in range(NOT):
                    # g before h: the sigmoid only needs g, so it starts
                    # while the h matmuls are still streaming
                    gp = gps_pool.tile([P, CH], F32, tag="gps")
                    for dt in range(NDT):
                        nc.tensor.matmul(
                            gp,
                            lhsT=gwT_sb[dt][:, ot * P:(ot + 1) * P],
                            rhs=c_sb[i][dt],
                            start=(dt == 0), stop=(dt == NDT - 1),
                        )
                    hp = hps_pool.tile([P, CH], F32, tag="hps")
                    for dt in range(NDT):
                        nc.tensor.matmul(
                            hp,
                            lhsT=hwT_sb[dt][:, ot * P:(ot + 1) * P],
                            rhs=c_sb[i][dt],
                            start=(dt == 0), stop=(dt == NDT - 1),
                        )
                    # a = sigmoid(-(g + bias))
                    at = apool.tile([P, CH], BF16, tag="at")
                    nc.scalar.activation(at, gp, AF.Sigmoid,
                                         bias=gbn_sb[ot], scale=-1.0)
                    # vhat = (a - 1) * h  == -(1-a)*h, h read from PSUM
                    nv = vpool.tile([P, CH], BF16, tag="nv")
                    nc.vector.scalar_tensor_tensor(nv, at, 1.0, hp,
                                                   OP.subtract, OP.mult)
                    # out = a*prev - vhat
                    ott = opool.tile([P, CH], BF16, tag=f"out{ot}")
                    init = 0.0 if i == 0 else prev_out[ot][:, CH - 1:CH]
                    nc.vector.tensor_tensor_scan(ott, at, nv, init,
                                                 OP.mult, OP.subtract)
                    if last:
                        # short transfers drain the tail faster
                        nc.sync.dma_start(
                            out=out[ot * P:(ot + 1) * P, lo:lo + CH // 2],
                            in_=ott[:, :CH // 2])
                        nc.sync.dma_start(
                            out=out[ot * P:(ot + 1) * P,
                                    lo + CH // 2:lo + CH],
                            in_=ott[:, CH // 2:])
                    else:
                        nc.sync.dma_start(
                            out=out[ot * P:(ot + 1) * P, lo:lo + CH],
                            in_=ott)
                    prev_out[ot] = ott

            # PE warm-up: dummy matmuls during the initial DMA wait trip the
            # HAM clock gate to 2.4 GHz before real work arrives; the result
            # (zeros) lands in an out region the chunk-0 store overwrites.
            wps = cps_pool.tile([P, CH], F32, tag="cps", name="warmps")
            for _ in range(28):
                nc.tensor.matmul(wps[0:2, 0:2], lhsT=warm_sb, rhs=warm_sb,
                                 start=True, stop=True)
            wout = wpool.tile([2, 2], BF16, tag="warmout")
            nc.vector.tensor_copy(wout, wps[0:2, 0:2])
            nc.gpsimd.dma_start(out=out[2:4, 0:2], in_=wout)

            # chunks, software-pipelined one chunk ahead
            emit_conv(0)
            emit_conv(1)
            for i in range(NCH - 2):
                emit_rest(i)
                emit_conv(i + 2)
            emit_rest(NCH - 2)
            emit_rest(NCH - 1, last=True)

    nc.finalize()
    return nc


_PROGRAM = None


def _get_program():
    global _PROGRAM
    if _PROGRAM is None:
        _PROGRAM = build_program()
    return _PROGRAM


def prepare_in_maps(x, conv_w, h_w, g_w):
    bf = ml_dtypes.bfloat16
    x = np.asarray(x, dtype=np.float32)
    conv_w = np.asarray(conv_w, dtype=np.float32)
    h_w = np.asarray(h_w, dtype=np.float32)
    g_w = np.asarray(g_w, dtype=np.float32)

    xpad = np.zeros((B, D, L + 4), np.float32)
    xpad[:, :, 2:L + 2] = x
    xpad = np.ascontiguousarray(xpad.astype(bf))

    hwT = np.ascontiguousarray(h_w[:, :, 0].T.astype(bf))         # [D, O]
    gw_pad = np.zeros((O, D), np.float32)
    gw_pad[2:, :] = g_w[:, :, 0]
    gwT = np.ascontiguousarray(gw_pad.T.astype(bf))               # [D, O]

    # diagonal conv-weight matrices for the PE d-tiles: [NPE*128, 5*128]
    cwdiag = np.zeros((NPE * P, NTAPS * P), np.float32)
    for dt in range(NPE):
        for k in range(NTAPS):
            blk = cwdiag[dt * P:(dt + 1) * P, k * P:(k + 1) * P]
            np.fill_diagonal(blk, conv_w[dt * P:(dt + 1) * P, 0, k])
    cwdiag = np.ascontiguousarray(cwdiag.astype(bf))

    # per-partition tap weights for the engine d-tiles (ACT scale input):
    # cols 0-4 for d-tile 3, cols 5-9 for d-tile 2 (odd chunks)
    wtap = np.ascontiguousarray(np.concatenate(
        [conv_w[3 * P:4 * P, 0, :], conv_w[2 * P:3 * P, 0, :]], axis=1))

    gbp = np.zeros((O, 1), np.float32)
    gbp[0, 0], gbp[1, 0] = -1000.0, 1000.0
    gbn = np.ascontiguousarray(-gbp)

    zpad = np.zeros((P, 2), bf)
    return [
        {"x": xpad[b], "hwT": hwT, "gwT": gwT, "cwdiag": cwdiag,
         "wtap": wtap, "gbn": gbn, "zpad": zpad}
        for b in range(B)
    ]


def kernel(x, conv_w, h_w, g_w):
    in_maps = prepare_in_maps(x, conv_w, h_w, g_w)
    nc = _get_program()
    res = run_bass_kernel_spmd(nc, in_maps, list(range(N_CORES))).results
    return np.stack([res[b]["out"].astype(np.float32) for b in range(B)],
                    axis=0)


# revision 17
# speedup vs baseline: 1.0698x; 1.0053x over previous
"""MinGRU Trainium2 kernel (bf16 data path, conv split PE/engines).

Reference computation (per batch b):
    c = depthwise_conv1d(x, conv_w, taps=5, pad=2)        # [D, L]
    h = h_w @ c                                           # [O, L]
    g = concat([-1000, +1000], g_w @ c)                   # [O, L]
    a = sigmoid(-g); out[l] = a[l]*out[l-1] + (1-a[l])*h[l]

Strategy: pure data-parallel over B (8 batches -> 8 NeuronCores), streamed
in 8 l-chunks of 512.  Everything below f32-PSUM precision runs in bf16
(x, weights, c, a, v-hat, out); the DVE scan keeps fp32 internal state so
bf16 operands only cost I/O rounding (~0.6% rel err total, gate is 2e-2).

Per chunk:
  - conv: d-tiles 0-2 as 5 diagonal bf16 matmuls on PE (PSUM f32), cast to
    bf16 SBUF on DVE/ACT; d-tile 3 on engines: 5 ACT scaled copies
    (per-partition conv weight as activation scale) + 4 GpSimd adds.
  - h/g 1x1 convs: bf16 matmuls on PE.
  - a = sigmoid(-(g+bias)) on ACT (bias rows 0/1 carry the -/+1000
    polarization; gw rows 0/1 are zero-padded).
  - vhat = (a-1)*h in ONE DVE scalar_tensor_tensor op (reads h from PSUM).
  - scan: out = a*prev - vhat on DVE (== a*prev + (1-a)*h).
    Channel 0 (a==1, vhat==0) stays exactly 0; channel 1 (a==0) becomes h;
    no special end-pass needed at the 2e-2 gate.
Output is stored bf16 and upcast to f32 on the host.
"""

import numpy as np
import ml_dtypes

import concourse.bass as bass
import concourse.mybir as mybir
from concourse import bacc
from concourse.tile import TileContext
from concourse.bass_utils import run_bass_kernel_spmd

BF16 = mybir.dt.bfloat16
F32 = mybir.dt.float32
AF = mybir.ActivationFunctionType
OP = mybir.AluOpType

B, D, O, L = 8, 512, 512, 4096
P = 128
CH = 512                 # l-chunk width (one PSUM bank)
NCH = L // CH            # 8
NDT = D // P             # 4 d-tiles
NOT = O // P             # 4 o-tiles
NTAPS = 5
N_CORES = 8
NPE = 3                  # conv d-tiles computed on PE; d-tile 3 on engines


def build_program():
    nc = bacc.Bacc()

    # x is host-padded with 2 zero columns on each side -> no halo cases
    x = nc.declare_dram_parameter("x", [D, L + 4], BF16, isOutput=False)
    hwT = nc.declare_dram_parameter("hwT", [D, O], BF16, isOutput=False)
    gwT = nc.declare_dram_parameter("gwT", [D, O], BF16, isOutput=False)
    cwdiag = nc.declare_dram_parameter("cwdiag", [NPE * P, NTAPS * P], BF16,
                                       isOutput=False)
    wtap = nc.declare_dram_parameter("wtap", [P, 2 * NTAPS], F32,
                                     isOutput=False)
    gbn = nc.declare_dram_parameter("gbn", [O, 1], F32, isOutput=False)
    zpad = nc.declare_dram_parameter("zpad", [P, 2], BF16, isOutput=False)
    out = nc.declare_dram_parameter("out", [O, L], BF16, isOutput=True)

    with TileContext(nc) as tc:
        with (
            tc.tile_pool(name="weights", bufs=1) as wpool,
            tc.tile_pool(name="xin", bufs=8) as xpool,
            tc.tile_pool(name="csb", bufs=8) as cpool,
            tc.tile_pool(name="mtap", bufs=10) as mpool,
            tc.tile_pool(name="actout", bufs=4) as apool,
            tc.tile_pool(name="vhat", bufs=4) as vpool,
            tc.tile_pool(name="outt", bufs=8) as opool,
            tc.tile_pool(name="cps", bufs=4, space="PSUM") as cps_pool,
            tc.tile_pool(name="hps", bufs=2, space="PSUM") as hps_pool,
            tc.tile_pool(name="gps", bufs=2, space="PSUM") as gps_pool,
        ):
            # tiny zero tile leads the scalar queue for PE warm-up matmuls
            warm_sb = wpool.tile([P, 2], BF16, tag="warm")
            nc.scalar.dma_start(out=warm_sb, in_=zpad[:, :])

            cw_sb = []
            for dt in range(NPE):
                t = wpool.tile([P, NTAPS * P], BF16, tag=f"cw{dt}")
                nc.scalar.dma_start(out=t, in_=cwdiag[dt * P:(dt + 1) * P, :])
                cw_sb.append(t)
            wtap_sb = wpool.tile([P, 2 * NTAPS], F32, tag="wtap")
            nc.scalar.dma_start(out=wtap_sb, in_=wtap[:, :])
            gbn_sb = []
            for ot in range(NOT):
                t = wpool.tile([P, 1], F32, tag=f"gbn{ot}")
                nc.scalar.dma_start(out=t, in_=gbn[ot * P:(ot + 1) * P, :])
                gbn_sb.append(t)

            # h/g weights: one coarse DMA per d-tile.  g weights go through
            # the GpSimd SWDGE queue (idle at preload time, few triggers so
            # chunk-0 conv adds aren't delayed); h weights on scalar HWDGE.
            hwT_sb, gwT_sb = [], []
            for dt in range(NDT):
                gt = wpool.tile([P, O], BF16, tag=f"gwT{dt}", name=f"gwT{dt}")
                nc.gpsimd.dma_start(out=gt, in_=gwT[dt * P:(dt + 1) * P, :])
                gwT_sb.append(gt)
            for dt in range(NDT):
                ht = wpool.tile([P, O], BF16, tag=f"hwT{dt}", name=f"hwT{dt}")
                nc.scalar.dma_start(out=ht, in_=hwT[dt * P:(dt + 1) * P, :])
                hwT_sb.append(ht)

            c_sb = [None] * NCH          # [chunk] -> list of 4 SBUF c tiles
            prev_out = [None] * NOT      # previous chunk's out tiles per ot

            def emit_conv(i):
                lo = i * CH
                tiles = []
                for dt in range(NDT):
                    xt = xpool.tile([P, CH + 4], BF16, tag="xt")
                    xq = nc.sync if dt % 2 == 0 else nc.scalar
                    xq.dma_start(out=xt[:, :],
                                 in_=x[dt * P:(dt + 1) * P, lo:lo + CH + 4])
                    if dt < NPE:
                        # PE path: 5 diagonal matmuls, center tap first
                        cp = cps_pool.tile([P, CH], F32, tag="cps")
                        tap_order = (2, 0, 1, 3, 4)
                        for j, k in enumerate(tap_order):
                            nc.tensor.matmul(
                                cp,
                                lhsT=cw_sb[dt][:, k * P:(k + 1) * P],
                                rhs=xt[:, k:k + CH],
                                start=(j == 0), stop=(j == NTAPS - 1),
                            )
                        ct = cpool.tile([P, CH], BF16, tag="ct")
                        if dt < 2:
                            nc.vector.tensor_copy(ct, cp)
                        else:
                            nc.scalar.copy(ct, cp)
                        tiles.append(ct)
                    else:
                        # engine path: ACT per-partition scaled copies + adds
                        base = 0 if dt == 3 else NTAPS
                        m = []
                        for k in range(NTAPS):
                            mt = mpool.tile([P, CH], BF16, tag="mt")
                            nc.scalar.activation(mt, xt[:, k:k + CH], AF.Copy,
                                                 bias=0.0,
                                                 scale=wtap_sb[:, base + k:
                                                               base + k + 1])
                            m.append(mt)
                        s01 = mpool.tile([P, CH], BF16, tag="s01")
                        nc.gpsimd.tensor_tensor(s01, m[0], m[1], OP.add)
                        s23 = mpool.tile([P, CH], BF16, tag="s23")
                        nc.gpsimd.tensor_tensor(s23, m[2], m[3], OP.add)
                        s014 = mpool.tile([P, CH], BF16, tag="s014")
                        nc.gpsimd.tensor_tensor(s014, s01, m[4], OP.add)
                        ct = cpool.tile([P, CH], BF16, tag="ct")
                        nc.gpsimd.tensor_tensor(ct, s014, s23, OP.add)
                        tiles.append(ct)
                c_sb[i] = tiles

            def emit_rest(i, last=False):
                lo = i * CH
                for ot in range(NOT):
                    # g before h: the sigmoid only needs g, so it starts
                    # while the h matmuls are still streaming
                    gp = gps_pool.tile([P, CH], F32, tag="gps")
                    for dt in range(NDT):
                        nc.tensor.matmul(
                            gp,
                            lhsT=gwT_sb[dt][:, ot * P:(ot + 1) * P],
                            rhs=c_sb[i][dt],
                            start=(dt == 0), stop=(dt == NDT - 1),
                        )
                    hp = hps_pool.tile([P, CH], F32, tag="hps")
                    for dt in range(NDT):
                        nc.tensor.matmul(
                            hp,
                            lhsT=hwT_sb[dt][:, ot * P:(ot + 1) * P],
                            rhs=c_sb[i][dt],
                            start=(dt == 0), stop=(dt == NDT - 1),
                        )
                    # a = sigmoid(-(g + bias))
                    at = apool.tile([P, CH], BF16, tag="at")
                    nc.scalar.activation(at, gp, AF.Sigmoid,
                                         bias=gbn_sb[ot], scale=-1.0)
                    # vhat = (a - 1) * h  == -(1-a)*h, h read from PSUM
                    nv = vpool.tile([P, CH], BF16, tag="nv")
                    nc.vector.scalar_tensor_tensor(nv, at, 1.0, hp,
                                                   OP.subtract, OP.mult)
                    # out = a*prev - vhat
                    ott = opool.tile([P, CH], BF16, tag=f"out{ot}")
                    init = 0.0 if i == 0 else prev_out[ot][:, CH - 1:CH]
                    nc.vector.tensor_tensor_scan(ott, at, nv, init,
                                                 OP.mult, OP.subtract)
                    if last:
                        # short transfers drain the tail faster
                        nc.sync.dma_start(
                            out=out[ot * P:(ot + 1) * P, lo:lo + CH // 2],
                            in_=ott[:, :CH // 2])
                        nc.sync.dma_start(
                            out=out[ot * P:(ot + 1) * P,
                                    lo + CH // 2:lo + CH],
                            in_=ott[:, CH // 2:])
                    else:
                        nc.sync.dma_start(
                            out=out[ot * P:(ot + 1) * P, lo:lo + CH],
                            in_=ott)
                    prev_out[ot] = ott

            # PE warm-up: dummy matmuls during the initial DMA wait trip the
            # HAM clock gate to 2.4 GHz before real work arrives; the result
            # (zeros) lands in an out region the chunk-0 store overwrites.
            warm_wide = wpool.tile([P, P], BF16, tag="warmw")
            nc.vector.memset(warm_wide, 0.0)
            wps = cps_pool.tile([P, CH], F32, tag="cps", name="warmps")
            for _ in range(28):
                nc.tensor.matmul(wps[0:2, 0:P], lhsT=warm_sb, rhs=warm_wide,
                                 start=True, stop=True)
            wout = wpool.tile([2, 2], BF16, tag="warmout")
            nc.vector.tensor_copy(wout, wps[0:2, 0:2])
            nc.gpsimd.dma_start(out=out[2:4, 0:2], in_=wout)

            # chunks, software-pipelined one chunk ahead
            emit_conv(0)
            emit_conv(1)
            for i in range(NCH - 2):
                emit_rest(i)
                emit_conv(i + 2)
            emit_rest(NCH - 2)
            emit_rest(NCH - 1, last=True)

    nc.finalize()
    return nc


_PROGRAM = None


def _get_program():
    global _PROGRAM
    if _PROGRAM is None:
        _PROGRAM = build_program()
    return _PROGRAM


def prepare_in_maps(x, conv_w, h_w, g_w):
    bf = ml_dtypes.bfloat16
    x = np.asarray(x, dtype=np.float32)
    conv_w = np.asarray(conv_w, dtype=np.float32)
    h_w = np.asarray(h_w, dtype=np.float32)
    g_w = np.asarray(g_w, dtype=np.float32)

    xpad = np.zeros((B, D, L + 4), np.float32)
    xpad[:, :, 2:L + 2] = x
    xpad = np.ascontiguousarray(xpad.astype(bf))

    hwT = np.ascontiguousarray(h_w[:, :, 0].T.astype(bf))         # [D, O]
    gw_pad = np.zeros((O, D), np.float32)
    gw_pad[2:, :] = g_w[:, :, 0]
    gwT = np.ascontiguousarray(gw_pad.T.astype(bf))               # [D, O]

    # diagonal conv-weight matrices for the PE d-tiles: [NPE*128, 5*128]
    cwdiag = np.zeros((NPE * P, NTAPS * P), np.float32)
    for dt in range(NPE):
        for k in range(NTAPS):
            blk = cwdiag[dt * P:(dt + 1) * P, k * P:(k + 1) * P]
            np.fill_diagonal(blk, conv_w[dt * P:(dt + 1) * P, 0, k])
    cwdiag = np.ascontiguousarray(cwdiag.astype(bf))

    # per-partition tap weights for the engine d-tiles (ACT scale input):
    # cols 0-4 for d-tile 3, cols 5-9 for d-tile 2 (odd chunks)
    wtap = np.ascontiguousarray(np.concatenate(
        [conv_w[3 * P:4 * P, 0, :], conv_w[2 * P:3 * P, 0, :]], axis=1))

    gbp = np.zeros((O, 1), np.float32)
    gbp[0, 0], gbp[1, 0] = -1000.0, 1000.0
    gbn = np.ascontiguousarray(-gbp)

    zpad = np.zeros((P, 2), bf)
    return [
        {"x": xpad[b], "hwT": hwT, "gwT": gwT, "cwdiag": cwdiag,
         "wtap": wtap, "gbn": gbn, "zpad": zpad}
        for b in range(B)
    ]


def kernel(x, conv_w, h_w, g_w):
    in_maps = prepare_in_maps(x, conv_w, h_w, g_w)
    nc = _get_program()
    res = run_bass_kernel_spmd(nc, in_maps, list(range(N_CORES))).results
    return np.stack([res[b]["out"].astype(np.float32) for b in range(B)],
                    axis=0)
